# revision 36
# baseline (speedup 1.0000x reference)
"""Trainium2 Bass kernel: Mixture-of-Experts SwiGLU feed-forward.

Module: x:[4,2048,512] -> router top-2-of-8 (softmax over selected
logits) -> per-expert SwiGLU FFN (h=silu(x@W1)*(x@W3); y=h@W2) ->
weighted combine.

Sharding (expert-parallel, per the hint): the host computes the router
(cheap: 8192x512x8 matmul + top-2), dispatches each expert's tokens to
the core owning that expert (all-to-all dispatch by top-k expert id),
each of the 8 NeuronCores runs its expert's FFN over a fixed-capacity
token batch (capacity factor 1.0 = 2048 tokens), and the host applies
gate weights and scatter-adds the expert outputs back into the full
output (weighted all-to-all return). The few tokens past an expert's
capacity (load imbalance remainder, ~1% of traffic) are computed on
the host instead of being dropped.

On-device compute uses bf16 matmuls (full-rate on the TRN2 PE, ~5e-3
relative error vs the 2e-2 gate) with fp32 PSUM accumulation. bf16 is
matmul-legal directly, so DMA lands input bytes straight into the
matmul tiles -- no on-device casts -- and halves HBM traffic vs fp32.
Activations live transposed ([feature, token]) on device so every
matmul consumes naturally-laid-out weights as the stationary operand
and no on-device transposes are needed.

Tokens are processed in four 512-wide blocks (the moving-operand /
PSUM-bank limit). Weights are host-permuted hidden-major so each DMA
fetches exactly the 128-column weight block the next psum group needs,
and transfers are spread across the three DMA-issuing engines (sync /
scalar HWDGE, gpsimd SWDGE) and emitted as late as possible: dma_start
occupies the issuing engine for the whole transfer, and the Tile
scheduler bundles per-queue semaphore waits, so early matmuls must not
share a queue epoch with transfers they don't need.
"""

import os
import sys
import types

for _p in ("/opt/trn_rl_repo",):
    if os.path.isdir(_p) and _p not in sys.path:
        sys.path.insert(0, _p)

import numpy as np
import ml_dtypes

BF16 = ml_dtypes.bfloat16

# Problem dims (fixed by the nn.Module spec)
D = 512          # d_model
H = 1024         # ffn hidden
E = 8            # experts
TOPK = 2
T = 8192         # tokens = 4*2048
P = 128          # SBUF partitions
CAP = 2048       # per-expert token capacity (capacity factor 1.0)
NB = CAP // 512  # 4 token blocks of 512
DK = D // P      # 4 contraction chunks over d
MH = H // P      # 8 hidden chunks
N_CORES = 8

_compiled = {}
last_exec_time_ns = None
last_results = None


def _install_axon_trace_shim():
    """Make trace=True under axon survive images without antenv.axon_hooks."""
    try:
        import antenv  # noqa: F401
    except Exception:
        return
    try:
        from antenv import axon_hooks  # noqa: F401
        return  # real module present
    except Exception:
        pass
    try:
        import antenv
        boot_dir = "/root/.axon_site/trn_agent_boot"
        if os.path.isdir(boot_dir) and boot_dir not in sys.path:
            sys.path.insert(0, boot_dir)
        import trn_boot
        mod = types.ModuleType("antenv.axon_hooks")
        holder = {"hook": trn_boot._ntff_profile_via_ctypes("/opt/axon/libaxon_pjrt.so")}
        mod.set_axon_ntff_profile_hook = lambda h: holder.__setitem__("hook", h)
        mod.get_axon_ntff_profile_hook = lambda: holder["hook"]
        sys.modules["antenv.axon_hooks"] = mod
        antenv.axon_hooks = mod
    except Exception:
        pass


def _patch_upload_artifacts():
    """Artifact upload needs fishnet; degrade to the local dir if absent."""
    try:
        import concourse.bass_utils as bu
        orig = bu.upload_artifacts

        def safe_upload(tmpdir):
            try:
                return orig(tmpdir)
            except Exception:
                return tmpdir

        if getattr(bu.upload_artifacts, "__name__", "") != "safe_upload":
            bu.upload_artifacts = safe_upload
    except Exception:
        pass


def _build():
    from concourse import bacc, mybir
    import concourse.tile as tile

    f32 = mybir.dt.float32
    bf16 = mybir.dt.bfloat16
    BT = 512         # token block (moving operand / PSUM bank limit)
    NBLK = CAP // BT

    nc = bacc.Bacc(num_swdge_queues=1)
    # All inputs arrive host-permuted partition-major so every staging
    # transfer is a plain contiguous 2D slice with 2KB-per-partition lines
    # (full DMA throughput, one issue slot for a 256-512KB transfer):
    #   xT  [P, NBLK*DK*512]: (p, b, k, t)
    #   w1/w3 [P, MH*DK*128]: (p, m, k, c)  -- hidden-block-major
    #   w2  [P, MH*D]:        (p, m, d)     -- p is the hidden contraction
    xT = nc.declare_dram_parameter("xT", [P, NBLK * DK * BT], bf16, isOutput=False)
    w1 = nc.declare_dram_parameter("w1", [P, MH * DK * P], bf16, isOutput=False)
    w3 = nc.declare_dram_parameter("w3", [P, MH * DK * P], bf16, isOutput=False)
    w2 = nc.declare_dram_parameter("w2", [P, MH * D], bf16, isOutput=False)
    yT = nc.declare_dram_parameter("yT", [D, CAP], bf16, isOutput=True)

    with tile.TileContext(nc) as tc:
        with tc.tile_pool(name="wpool", bufs=1) as wpool, \
             tc.tile_pool(name="act", bufs=2) as act, \
             tc.tile_pool(name="psum", bufs=1, space="PSUM") as psum:

            w1s = wpool.tile([P, MH, DK * P], bf16, tag="w1s")
            w3s = wpool.tile([P, MH, DK * P], bf16, tag="w3s")
            w2s = wpool.tile([P, MH, D], bf16, tag="w2s")
            xs = wpool.tile([P, NBLK, DK, BT], bf16, tag="xs")

            # DMA engine-time is SYNCHRONOUS on the issuing engine, and a
            # tile reader's per-queue sem wait gets bundled with transfers
            # emitted near its own. So: few, big transfers; spread over
            # sync/gpsimd (idle engines) + scalar's head slack; emitted as
            # late as possible interleaved with compute emission.
            def stage_w(dst, src, m0, n, eng):
                eng.dma_start(out=dst[:, m0:m0 + n],
                              in_=src[:, m0 * DK * P:(m0 + n) * DK * P])

            # Minimal pre-loop set: exactly what the first psum groups
            # need, kept as SMALL single transfers (the first groups' sem
            # waits bundle with whole transfers; fine granularity keeps
            # the rounded-up wait cheap). Everything staged later uses
            # merged 2KB-line transfers.
            stage_w(w1s, w1, 0, 1, nc.sync)
            nc.sync.dma_start(out=xs[:, 0, 0], in_=xT[:, 0:BT])
            nc.sync.dma_start(out=xs[:, 0, 1], in_=xT[:, BT:2 * BT])
            nc.scalar.dma_start(out=xs[:, 0, 2], in_=xT[:, 2 * BT:3 * BT])
            nc.scalar.dma_start(out=xs[:, 0, 3], in_=xT[:, 3 * BT:4 * BT])
            stage_w(w3s, w3, 0, 2, nc.gpsimd)

            ht = wpool.tile([P, MH, BT], bf16, tag="ht")
            for blk in range(NBLK):
                for m in range(MH):
                    ps1 = psum.tile([P, BT], f32, tag="ps1", bufs=2)
                    ps2 = psum.tile([P, BT], f32, tag="ps2", bufs=2)
                    for k in range(DK):
                        nc.tensor.matmul(out=ps1[:], lhsT=w1s[:, m, k * P:(k + 1) * P],
                                         rhs=xs[:, blk, k],
                                         start=(k == 0), stop=(k == DK - 1))
                    sil = act.tile([P, BT], f32, tag="sil")
                    nc.scalar.activation(sil[:], ps1[:],
                                         mybir.ActivationFunctionType.Silu)
                    for k in range(DK):
                        nc.tensor.matmul(out=ps2[:], lhsT=w3s[:, m, k * P:(k + 1) * P],
                                         rhs=xs[:, blk, k],
                                         start=(k == 0), stop=(k == DK - 1))
                    nc.vector.tensor_mul(out=ht[:, m], in0=sil[:], in1=ps2[:])
                    # late staging, emitted right after its last blocker
                    if blk == 0:
                        if m == 0:
                            stage_w(w1s, w1, 1, 1, nc.sync)
                        elif m < 4:
                            stage_w(w1s, w1, 2 * m, 2, nc.sync)
                        if m < 3:
                            stage_w(w3s, w3, 2 * (m + 1), 2, nc.gpsimd)
                    if blk == 0 and 2 <= m < 6:
                        nc.sync.dma_start(
                            out=w2s[:, 2 * (m - 2):2 * (m - 1)],
                            in_=w2[:, 2 * (m - 2) * D:2 * (m - 1) * D])
                    if blk < NBLK - 1 and m == 2:
                        nc.scalar.dma_start(
                            out=xs[:, blk + 1],
                            in_=xT[:, (blk + 1) * DK * BT:(blk + 2) * DK * BT])
                tok = slice(BT * blk, BT * blk + BT)
                for j in range(DK):
                    js = slice(j * P, (j + 1) * P)
                    psy = psum.tile([P, BT], f32, tag="psy", bufs=2)
                    for m in range(MH):
                        nc.tensor.matmul(out=psy[:], lhsT=w2s[:, m, js],
                                         rhs=ht[:, m],
                                         start=(m == 0), stop=(m == MH - 1))
                    yt = act.tile([P, BT], bf16, tag="yt")
                    nc.vector.tensor_copy(out=yt[:], in_=psy[:])
                    # outputs: gpsimd is idle after the weight head, and
                    # using it keeps output completions out of the sem
                    # ranges x-staging consumers get bundled with. The last
                    # block's flush goes on the two fast HW queues.
                    if blk < NBLK - 1:
                        nc.gpsimd.dma_start(out=yT[js, tok], in_=yt[:])
                    elif j % 2 == 0:
                        nc.scalar.dma_start(out=yT[js, tok], in_=yt[:])
                    else:
                        nc.sync.dma_start(out=yT[js, tok], in_=yt[:])

    nc.compile()
    return nc


def _route(x2d, Wg, bg):
    """Replicate the reference router on host.

    Selection runs in float64 (agrees with the reference's fp32 jax
    selection whenever top-2/top-3 logit gaps exceed fp32 matmul noise,
    which holds with >10x margin on this distribution); the softmax over
    the two selected logits runs in fp32 like the reference.
    """
    logits64 = x2d.astype(np.float64) @ Wg.astype(np.float64) + bg.astype(np.float64)
    i1 = np.argmax(logits64, axis=1)
    r = np.arange(T)
    masked = logits64.copy()
    masked[r, i1] = -np.inf
    i2 = np.argmax(masked, axis=1)

    # fp32 logit values for the softmax (match reference arithmetic)
    logits32 = (x2d @ Wg + bg).astype(np.float32)
    v1 = logits32[r, i1]
    v2 = logits32[r, i2]
    # softmax over [v1, v2] with v1 >= v2 (fp32)
    e2 = np.exp((v2 - v1).astype(np.float32))
    p1 = (1.0 / (1.0 + e2)).astype(np.float32)
    p2 = (e2 / (1.0 + e2)).astype(np.float32)
    return i1, i2, p1, p2


def _ffn_host(x2d, idx, W1e, W3e, W2e):
    """Exact fp32 SwiGLU FFN for a small set of tokens (overflow path)."""
    z = x2d[idx] @ W1e
    h = (z / (1.0 + np.exp(-z))) * (x2d[idx] @ W3e)
    return h @ W2e


def kernel(x, Wg, bg, W1, W3, W2):
    global last_exec_time_ns
    _install_axon_trace_shim()
    _patch_upload_artifacts()
    from concourse.bass_utils import run_bass_kernel_spmd

    x = np.asarray(x, np.float32)
    Wg = np.asarray(Wg, np.float32)
    bg = np.asarray(bg, np.float32)
    W1 = np.asarray(W1, np.float32)
    W3 = np.asarray(W3, np.float32)
    W2 = np.asarray(W2, np.float32)

    B, S, _ = x.shape
    x2d = np.ascontiguousarray(x.reshape(T, D))

    i1, i2, p1, p2 = _route(x2d, Wg, bg)

    # Dispatch: build each expert's token list + gate weights. Tokens past
    # CAP (load-imbalance remainder) fall to the exact host path.
    idx_lists, gate_lists = [], []
    spill_lists = []
    for e in range(E):
        m1 = i1 == e
        m2 = i2 == e
        idx = np.concatenate([np.nonzero(m1)[0], np.nonzero(m2)[0]])
        g = np.concatenate([p1[m1], p2[m2]]).astype(np.float32)
        if len(idx) > CAP:
            # Spill the smallest-gate tokens: they matter least if anything
            # about the two paths' rounding ever differs.
            order = np.argsort(-g, kind="stable")
            idx, g = idx[order], g[order]
            spill_lists.append((idx[CAP:], g[CAP:]))
            idx, g = idx[:CAP], g[:CAP]
        else:
            spill_lists.append((idx[:0], g[:0]))
        idx_lists.append(idx)
        gate_lists.append(g)

    x2dT_bf = np.ascontiguousarray(x2d.T.astype(BF16))  # [D, T]
    in_maps = []
    for e in range(E):
        idx = idx_lists[e]
        xe = np.zeros((D, CAP), BF16)
        xe[:, : len(idx)] = x2dT_bf[:, idx]
        # partition-major device layouts (see _build for the index maps)
        xe_dev = np.ascontiguousarray(
            xe.reshape(DK, P, NB, CAP // NB).transpose(1, 2, 0, 3)
        ).reshape(P, CAP * DK)
        w1m = np.ascontiguousarray(
            W1[e].astype(BF16).reshape(DK, P, MH, P).transpose(1, 2, 0, 3)
        ).reshape(P, MH * DK * P)
        w3m = np.ascontiguousarray(
            W3[e].astype(BF16).reshape(DK, P, MH, P).transpose(1, 2, 0, 3)
        ).reshape(P, MH * DK * P)
        w2m = np.ascontiguousarray(
            W2[e].astype(BF16).reshape(MH, P, D).transpose(1, 0, 2)
        ).reshape(P, MH * D)
        in_maps.append({
            "xT": xe_dev,
            "w1": w1m,
            "w3": w3m,
            "w2": w2m,
        })

    if "nc" not in _compiled:
        _compiled["nc"] = _build()
    nc = _compiled["nc"]

    trace = bool(os.environ.get("BASS_TRACE"))
    res = run_bass_kernel_spmd(nc, in_maps, list(range(N_CORES)), trace=trace)
    last_exec_time_ns = res.exec_time_ns
    globals()["last_results"] = res

    y = np.zeros((T, D), np.float32)
    for e in range(E):
        idx = idx_lists[e]
        n = len(idx)
        ye = np.asarray(res.results[e]["yT"])  # [D, CAP] bf16
        y[idx] += gate_lists[e][:, None] * ye[:, :n].T.astype(np.float32)
        sidx, sg = spill_lists[e]
        if len(sidx):
            y[sidx] += sg[:, None] * _ffn_host(x2d, sidx, W1[e], W3[e], W2[e])
    return y.reshape(B, S, D)


# revision 37
# speedup vs baseline: 1.0539x; 1.0539x over previous
"""Trainium2 Bass kernel: Mixture-of-Experts SwiGLU feed-forward.

Module: x:[4,2048,512] -> router top-2-of-8 (softmax over selected
logits) -> per-expert SwiGLU FFN (h=silu(x@W1)*(x@W3); y=h@W2) ->
weighted combine.

Sharding (expert-parallel, per the hint): the host computes the router
(cheap: 8192x512x8 matmul + top-2), dispatches each expert's tokens to
the core owning that expert (all-to-all dispatch by top-k expert id),
each of the 8 NeuronCores runs its expert's FFN over a fixed-capacity
token batch (capacity factor 1.0 = 2048 tokens), and the host applies
gate weights and scatter-adds the expert outputs back into the full
output (weighted all-to-all return). The few tokens past an expert's
capacity (load imbalance remainder, ~1% of traffic) are computed on
the host instead of being dropped.

On-device compute uses bf16 matmuls (full-rate on the TRN2 PE, ~5e-3
relative error vs the 2e-2 gate) with fp32 PSUM accumulation. bf16 is
matmul-legal directly, so DMA lands input bytes straight into the
matmul tiles -- no on-device casts -- and halves HBM traffic vs fp32.
Activations live transposed ([feature, token]) on device so every
matmul consumes naturally-laid-out weights as the stationary operand
and no on-device transposes are needed.

Tokens are processed in four 512-wide blocks (the moving-operand /
PSUM-bank limit). Weights are host-permuted hidden-major so each DMA
fetches exactly the 128-column weight block the next psum group needs,
and transfers are spread across the three DMA-issuing engines (sync /
scalar HWDGE, gpsimd SWDGE) and emitted as late as possible: dma_start
occupies the issuing engine for the whole transfer, and the Tile
scheduler bundles per-queue semaphore waits, so early matmuls must not
share a queue epoch with transfers they don't need.
"""

import os
import sys
import types

for _p in ("/opt/trn_rl_repo",):
    if os.path.isdir(_p) and _p not in sys.path:
        sys.path.insert(0, _p)

import numpy as np
import ml_dtypes

BF16 = ml_dtypes.bfloat16

# Problem dims (fixed by the nn.Module spec)
D = 512          # d_model
H = 1024         # ffn hidden
E = 8            # experts
TOPK = 2
T = 8192         # tokens = 4*2048
P = 128          # SBUF partitions
CAP = 2048       # per-expert token capacity (capacity factor 1.0)
NB = CAP // 512  # 4 token blocks of 512
DK = D // P      # 4 contraction chunks over d
MH = H // P      # 8 hidden chunks
N_CORES = 8

_compiled = {}
last_exec_time_ns = None
last_results = None


def _install_axon_trace_shim():
    """Make trace=True under axon survive images without antenv.axon_hooks."""
    try:
        import antenv  # noqa: F401
    except Exception:
        return
    try:
        from antenv import axon_hooks  # noqa: F401
        return  # real module present
    except Exception:
        pass
    try:
        import antenv
        boot_dir = "/root/.axon_site/trn_agent_boot"
        if os.path.isdir(boot_dir) and boot_dir not in sys.path:
            sys.path.insert(0, boot_dir)
        import trn_boot
        mod = types.ModuleType("antenv.axon_hooks")
        holder = {"hook": trn_boot._ntff_profile_via_ctypes("/opt/axon/libaxon_pjrt.so")}
        mod.set_axon_ntff_profile_hook = lambda h: holder.__setitem__("hook", h)
        mod.get_axon_ntff_profile_hook = lambda: holder["hook"]
        sys.modules["antenv.axon_hooks"] = mod
        antenv.axon_hooks = mod
    except Exception:
        pass


def _patch_upload_artifacts():
    """Artifact upload needs fishnet; degrade to the local dir if absent."""
    try:
        import concourse.bass_utils as bu
        orig = bu.upload_artifacts

        def safe_upload(tmpdir):
            try:
                return orig(tmpdir)
            except Exception:
                return tmpdir

        if getattr(bu.upload_artifacts, "__name__", "") != "safe_upload":
            bu.upload_artifacts = safe_upload
    except Exception:
        pass


def _build():
    from concourse import bacc, mybir
    import concourse.tile as tile

    f32 = mybir.dt.float32
    bf16 = mybir.dt.bfloat16
    BT = 512         # token block (moving operand / PSUM bank limit)
    NBLK = CAP // BT

    nc = bacc.Bacc(num_swdge_queues=1)
    xT = nc.declare_dram_parameter("xT", [D, CAP], bf16, isOutput=False)
    # w1/w3 arrive host-permuted m-major: [MH*P, DK*128] so one DMA fetches
    # exactly the hidden-column block the next psum group needs.
    w1 = nc.declare_dram_parameter("w1", [MH * P, DK * P], bf16, isOutput=False)
    w3 = nc.declare_dram_parameter("w3", [MH * P, DK * P], bf16, isOutput=False)
    w2 = nc.declare_dram_parameter("w2", [H, D], bf16, isOutput=False)
    yT = nc.declare_dram_parameter("yT", [D, CAP], bf16, isOutput=True)

    with tile.TileContext(nc) as tc:
        with tc.tile_pool(name="wpool", bufs=1) as wpool, \
             tc.tile_pool(name="act", bufs=2) as act, \
             tc.tile_pool(name="psum", bufs=1, space="PSUM") as psum:

            w1s = wpool.tile([P, MH, DK * P], bf16, tag="w1s")
            w3s = wpool.tile([P, MH, DK * P], bf16, tag="w3s")
            w2s = wpool.tile([P, MH, D], bf16, tag="w2s")
            xs = wpool.tile([P, DK, CAP], bf16, tag="xs")

            w1v = w1[:].rearrange("(m p) c -> m p c", p=P)
            w3v = w3[:].rearrange("(m p) c -> m p c", p=P)
            w2v = w2[:].rearrange("(k p) d -> p k d", p=P)
            xv = xT[:].rearrange("(k p) t -> p k t", p=P)

            # DMA engine-time is SYNCHRONOUS on the issuing engine (~600ns
            # per 128-256KB transfer), and a tile reader waits on every
            # write to that tile emitted so far. So: (a) spread transfers
            # over sync/gpsimd (idle engines) + a little scalar slack, and
            # (b) emit each transfer as late as possible, interleaved with
            # compute emission, so early matmuls don't wait on late writes.
            def stage_x(b, k, eng):
                eng.dma_start(out=xs[:, k, 512 * b:512 * (b + 1)],
                              in_=xv[:, k, 512 * b:512 * (b + 1)])

            # minimal pre-loop set: exactly what the first psum groups need
            nc.sync.dma_start(out=w1s[:, 0], in_=w1v[0])
            stage_x(0, 0, nc.sync)
            stage_x(0, 1, nc.sync)
            stage_x(0, 2, nc.scalar)
            stage_x(0, 3, nc.scalar)
            nc.gpsimd.dma_start(out=w3s[:, 0], in_=w3v[0])
            nc.gpsimd.dma_start(out=w3s[:, 1], in_=w3v[1])

            ht = wpool.tile([P, MH, BT], bf16, tag="ht")
            for blk in range(NBLK):
                tok = slice(BT * blk, BT * blk + BT)
                for m in range(MH):
                    ps1 = psum.tile([P, BT], f32, tag="ps1", bufs=2)
                    ps2 = psum.tile([P, BT], f32, tag="ps2", bufs=2)
                    for k in range(DK):
                        nc.tensor.matmul(out=ps1[:], lhsT=w1s[:, m, k * P:(k + 1) * P],
                                         rhs=xs[:, k, tok],
                                         start=(k == 0), stop=(k == DK - 1))
                    sil = act.tile([P, BT], f32, tag="sil")
                    nc.scalar.activation(sil[:], ps1[:],
                                         mybir.ActivationFunctionType.Silu)
                    for k in range(DK):
                        nc.tensor.matmul(out=ps2[:], lhsT=w3s[:, m, k * P:(k + 1) * P],
                                         rhs=xs[:, k, tok],
                                         start=(k == 0), stop=(k == DK - 1))
                    nc.vector.tensor_mul(out=ht[:, m], in0=sil[:], in1=ps2[:])
                    # Late staging, emitted right after its last blocker.
                    # Queue choice keeps each queue's head short: sem waits
                    # get bundled per queue by the scheduler, so a consumer
                    # can end up waiting on transfers emitted shortly after
                    # its own (keep those cheap or on other queues).
                    if blk == 0:
                        if m < MH - 1:
                            nc.sync.dma_start(out=w1s[:, m + 1], in_=w1v[m + 1])
                        if m < MH - 2:
                            nc.gpsimd.dma_start(out=w3s[:, m + 2], in_=w3v[m + 2])
                        if m >= 1:
                            nc.sync.dma_start(out=w2s[:, m - 1], in_=w2v[:, m - 1])
                        if m == MH - 1:
                            nc.sync.dma_start(out=w2s[:, 7], in_=w2v[:, 7])
                    if blk < NBLK - 1 and 1 <= m < 1 + DK:
                        stage_x(blk + 1, m - 1, nc.gpsimd if m % 2 else nc.sync)
                for j in range(DK):
                    js = slice(j * P, (j + 1) * P)
                    psy = psum.tile([P, BT], f32, tag="psy", bufs=2)
                    for m in range(MH):
                        nc.tensor.matmul(out=psy[:], lhsT=w2s[:, m, js],
                                         rhs=ht[:, m],
                                         start=(m == 0), stop=(m == MH - 1))
                    yt = act.tile([P, BT], bf16, tag="yt")
                    nc.vector.tensor_copy(out=yt[:], in_=psy[:])
                    # outputs: gpsimd is idle after the weight head, and
                    # using it keeps output completions out of the sem
                    # ranges x-staging consumers get bundled with. The last
                    # block's flush goes on the two fast HW queues.
                    if blk < NBLK - 1:
                        nc.gpsimd.dma_start(out=yT[js, tok], in_=yt[:])
                    elif j % 2 == 0:
                        nc.scalar.dma_start(out=yT[js, tok], in_=yt[:])
                    else:
                        nc.sync.dma_start(out=yT[js, tok], in_=yt[:])

    nc.compile()
    return nc


def _route(x2d, Wg, bg):
    """Replicate the reference router on host.

    Selection runs in float64 (agrees with the reference's fp32 jax
    selection whenever top-2/top-3 logit gaps exceed fp32 matmul noise,
    which holds with >10x margin on this distribution); the softmax over
    the two selected logits runs in fp32 like the reference.
    """
    logits64 = x2d.astype(np.float64) @ Wg.astype(np.float64) + bg.astype(np.float64)
    i1 = np.argmax(logits64, axis=1)
    r = np.arange(T)
    masked = logits64.copy()
    masked[r, i1] = -np.inf
    i2 = np.argmax(masked, axis=1)

    # fp32 logit values for the softmax (match reference arithmetic)
    logits32 = (x2d @ Wg + bg).astype(np.float32)
    v1 = logits32[r, i1]
    v2 = logits32[r, i2]
    # softmax over [v1, v2] with v1 >= v2 (fp32)
    e2 = np.exp((v2 - v1).astype(np.float32))
    p1 = (1.0 / (1.0 + e2)).astype(np.float32)
    p2 = (e2 / (1.0 + e2)).astype(np.float32)
    return i1, i2, p1, p2


def _ffn_host(x2d, idx, W1e, W3e, W2e):
    """Exact fp32 SwiGLU FFN for a small set of tokens (overflow path)."""
    z = x2d[idx] @ W1e
    h = (z / (1.0 + np.exp(-z))) * (x2d[idx] @ W3e)
    return h @ W2e


def kernel(x, Wg, bg, W1, W3, W2):
    global last_exec_time_ns
    _install_axon_trace_shim()
    _patch_upload_artifacts()
    from concourse.bass_utils import run_bass_kernel_spmd

    x = np.asarray(x, np.float32)
    Wg = np.asarray(Wg, np.float32)
    bg = np.asarray(bg, np.float32)
    W1 = np.asarray(W1, np.float32)
    W3 = np.asarray(W3, np.float32)
    W2 = np.asarray(W2, np.float32)

    B, S, _ = x.shape
    x2d = np.ascontiguousarray(x.reshape(T, D))

    i1, i2, p1, p2 = _route(x2d, Wg, bg)

    # Dispatch: build each expert's token list + gate weights. Tokens past
    # CAP (load-imbalance remainder) fall to the exact host path.
    idx_lists, gate_lists = [], []
    spill_lists = []
    for e in range(E):
        m1 = i1 == e
        m2 = i2 == e
        idx = np.concatenate([np.nonzero(m1)[0], np.nonzero(m2)[0]])
        g = np.concatenate([p1[m1], p2[m2]]).astype(np.float32)
        if len(idx) > CAP:
            # Spill the smallest-gate tokens: they matter least if anything
            # about the two paths' rounding ever differs.
            order = np.argsort(-g, kind="stable")
            idx, g = idx[order], g[order]
            spill_lists.append((idx[CAP:], g[CAP:]))
            idx, g = idx[:CAP], g[:CAP]
        else:
            spill_lists.append((idx[:0], g[:0]))
        idx_lists.append(idx)
        gate_lists.append(g)

    x2dT_bf = np.ascontiguousarray(x2d.T.astype(BF16))  # [D, T]
    in_maps = []
    for e in range(E):
        idx = idx_lists[e]
        xe = np.zeros((D, CAP), BF16)
        xe[:, : len(idx)] = x2dT_bf[:, idx]
        # m-major weight layout: [MH, P, DK*128] flattened to [MH*P, DK*128]
        w1m = np.ascontiguousarray(
            W1[e].astype(BF16).reshape(DK, P, MH, P).transpose(2, 1, 0, 3)
        ).reshape(MH * P, DK * P)
        w3m = np.ascontiguousarray(
            W3[e].astype(BF16).reshape(DK, P, MH, P).transpose(2, 1, 0, 3)
        ).reshape(MH * P, DK * P)
        in_maps.append({
            "xT": xe,
            "w1": w1m,
            "w3": w3m,
            "w2": np.ascontiguousarray(W2[e].astype(BF16)),
        })

    if "nc" not in _compiled:
        _compiled["nc"] = _build()
    nc = _compiled["nc"]

    trace = bool(os.environ.get("BASS_TRACE"))
    res = run_bass_kernel_spmd(nc, in_maps, list(range(N_CORES)), trace=trace)
    last_exec_time_ns = res.exec_time_ns
    globals()["last_results"] = res

    y = np.zeros((T, D), np.float32)
    for e in range(E):
        idx = idx_lists[e]
        n = len(idx)
        ye = np.asarray(res.results[e]["yT"])  # [D, CAP] bf16
        y[idx] += gate_lists[e][:, None] * ye[:, :n].T.astype(np.float32)
        sidx, sg = spill_lists[e]
        if len(sidx):
            y[sidx] += sg[:, None] * _ffn_host(x2d, sidx, W1[e], W3[e], W2[e])
    return y.reshape(B, S, D)


# revision 38
# speedup vs baseline: 1.0599x; 1.0057x over previous
"""Trainium2 Bass kernel: Mixture-of-Experts SwiGLU feed-forward.

Module: x:[4,2048,512] -> router top-2-of-8 (softmax over selected
logits) -> per-expert SwiGLU FFN (h=silu(x@W1)*(x@W3); y=h@W2) ->
weighted combine.

Sharding (expert-parallel, per the hint): the host computes the router
(cheap: 8192x512x8 matmul + top-2), dispatches each expert's tokens to
the core owning that expert (all-to-all dispatch by top-k expert id),
each of the 8 NeuronCores runs its expert's FFN over a fixed-capacity
token batch (capacity factor 1.0 = 2048 tokens), and the host applies
gate weights and scatter-adds the expert outputs back into the full
output (weighted all-to-all return). The few tokens past an expert's
capacity (load imbalance remainder, ~1% of traffic) are computed on
the host instead of being dropped.

On-device compute uses bf16 matmuls (full-rate on the TRN2 PE, ~5e-3
relative error vs the 2e-2 gate) with fp32 PSUM accumulation. bf16 is
matmul-legal directly, so DMA lands input bytes straight into the
matmul tiles -- no on-device casts -- and halves HBM traffic vs fp32.
Activations live transposed ([feature, token]) on device so every
matmul consumes naturally-laid-out weights as the stationary operand
and no on-device transposes are needed.

Tokens are processed in four 512-wide blocks (the moving-operand /
PSUM-bank limit). Weights are host-permuted hidden-major so each DMA
fetches exactly the 128-column weight block the next psum group needs,
and transfers are spread across the three DMA-issuing engines (sync /
scalar HWDGE, gpsimd SWDGE) and emitted as late as possible: dma_start
occupies the issuing engine for the whole transfer, and the Tile
scheduler bundles per-queue semaphore waits, so early matmuls must not
share a queue epoch with transfers they don't need.
"""

import os
import sys
import types

for _p in ("/opt/trn_rl_repo",):
    if os.path.isdir(_p) and _p not in sys.path:
        sys.path.insert(0, _p)

import numpy as np
import ml_dtypes

BF16 = ml_dtypes.bfloat16

# Problem dims (fixed by the nn.Module spec)
D = 512          # d_model
H = 1024         # ffn hidden
E = 8            # experts
TOPK = 2
T = 8192         # tokens = 4*2048
P = 128          # SBUF partitions
CAP = 2048       # per-expert token capacity (capacity factor 1.0)
NB = CAP // 512  # 4 token blocks of 512
DK = D // P      # 4 contraction chunks over d
MH = H // P      # 8 hidden chunks
N_CORES = 8

_compiled = {}
last_exec_time_ns = None
last_results = None


def _install_axon_trace_shim():
    """Make trace=True under axon survive images without antenv.axon_hooks."""
    try:
        import antenv  # noqa: F401
    except Exception:
        return
    try:
        from antenv import axon_hooks  # noqa: F401
        return  # real module present
    except Exception:
        pass
    try:
        import antenv
        boot_dir = "/root/.axon_site/trn_agent_boot"
        if os.path.isdir(boot_dir) and boot_dir not in sys.path:
            sys.path.insert(0, boot_dir)
        import trn_boot
        mod = types.ModuleType("antenv.axon_hooks")
        holder = {"hook": trn_boot._ntff_profile_via_ctypes("/opt/axon/libaxon_pjrt.so")}
        mod.set_axon_ntff_profile_hook = lambda h: holder.__setitem__("hook", h)
        mod.get_axon_ntff_profile_hook = lambda: holder["hook"]
        sys.modules["antenv.axon_hooks"] = mod
        antenv.axon_hooks = mod
    except Exception:
        pass


def _patch_upload_artifacts():
    """Artifact upload needs fishnet; degrade to the local dir if absent."""
    try:
        import concourse.bass_utils as bu
        orig = bu.upload_artifacts

        def safe_upload(tmpdir):
            try:
                return orig(tmpdir)
            except Exception:
                return tmpdir

        if getattr(bu.upload_artifacts, "__name__", "") != "safe_upload":
            bu.upload_artifacts = safe_upload
    except Exception:
        pass


def _build():
    from concourse import bacc, mybir
    import concourse.tile as tile

    f32 = mybir.dt.float32
    bf16 = mybir.dt.bfloat16
    BT = 512         # token block (moving operand / PSUM bank limit)
    NBLK = CAP // BT

    nc = bacc.Bacc(num_swdge_queues=1)
    xT = nc.declare_dram_parameter("xT", [D, CAP], bf16, isOutput=False)
    # w1/w3 arrive host-permuted m-major: [MH*P, DK*128] so one DMA fetches
    # exactly the hidden-column block the next psum group needs.
    w1 = nc.declare_dram_parameter("w1", [MH * P, DK * P], bf16, isOutput=False)
    w3 = nc.declare_dram_parameter("w3", [MH * P, DK * P], bf16, isOutput=False)
    w2 = nc.declare_dram_parameter("w2", [H, D], bf16, isOutput=False)
    yT = nc.declare_dram_parameter("yT", [D, CAP], bf16, isOutput=True)

    with tile.TileContext(nc) as tc:
        with tc.tile_pool(name="wpool", bufs=1) as wpool, \
             tc.tile_pool(name="act", bufs=2) as act, \
             tc.tile_pool(name="psum", bufs=1, space="PSUM") as psum:

            w1s = wpool.tile([P, MH, DK * P], bf16, tag="w1s")
            w3s = wpool.tile([P, MH, DK * P], bf16, tag="w3s")
            w2s = wpool.tile([P, MH, D], bf16, tag="w2s")
            xs = wpool.tile([P, DK, CAP], bf16, tag="xs")

            w1v = w1[:].rearrange("(m p) c -> m p c", p=P)
            w3v = w3[:].rearrange("(m p) c -> m p c", p=P)
            w2v = w2[:].rearrange("(k p) d -> p k d", p=P)
            xv = xT[:].rearrange("(k p) t -> p k t", p=P)

            # DMA engine-time is SYNCHRONOUS on the issuing engine (~600ns
            # per 128-256KB transfer), and a tile reader waits on every
            # write to that tile emitted so far. So: (a) spread transfers
            # over sync/gpsimd (idle engines) + a little scalar slack, and
            # (b) emit each transfer as late as possible, interleaved with
            # compute emission, so early matmuls don't wait on late writes.
            def stage_x(b, k, eng):
                eng.dma_start(out=xs[:, k, 512 * b:512 * (b + 1)],
                              in_=xv[:, k, 512 * b:512 * (b + 1)])

            # Minimal pre-loop set: exactly what the first psum groups need.
            # DMA data lands ~2us after issue (deep queue pipeline; the
            # completion sem posts 16 packet increments well after the
            # issue instruction retires), so the five critical transfers
            # sit first/second in THREE queues rather than 3-deep in one.
            nc.sync.dma_start(out=w1s[:, 0], in_=w1v[0])
            stage_x(0, 0, nc.scalar)
            stage_x(0, 1, nc.gpsimd)
            stage_x(0, 2, nc.sync)
            stage_x(0, 3, nc.scalar)
            nc.gpsimd.dma_start(out=w3s[:, 0], in_=w3v[0])
            nc.gpsimd.dma_start(out=w3s[:, 1], in_=w3v[1])

            # PE warmup: dummy matmuls (no DMA deps) fill the dead window
            # between engine init (~7us) and first operand arrival
            # (~9.6us), so the p-state ramp is underway when real work
            # starts.
            wscr = wpool.tile([P, P], bf16, tag="wscr")
            nc.vector.memset(wscr[:], 0)
            warm = psum.tile([P, P], f32, tag="warm")
            for _ in range(12):
                nc.tensor.matmul(out=warm[:], lhsT=wscr[:], rhs=wscr[:],
                                 start=True, stop=True)

            ht = wpool.tile([P, MH, BT], bf16, tag="ht")
            for blk in range(NBLK):
                tok = slice(BT * blk, BT * blk + BT)
                for m in range(MH):
                    ps1 = psum.tile([P, BT], f32, tag="ps1", bufs=2)
                    ps2 = psum.tile([P, BT], f32, tag="ps2", bufs=2)
                    for k in range(DK):
                        nc.tensor.matmul(out=ps1[:], lhsT=w1s[:, m, k * P:(k + 1) * P],
                                         rhs=xs[:, k, tok],
                                         start=(k == 0), stop=(k == DK - 1))
                    sil = act.tile([P, BT], f32, tag="sil")
                    nc.scalar.activation(sil[:], ps1[:],
                                         mybir.ActivationFunctionType.Silu)
                    for k in range(DK):
                        nc.tensor.matmul(out=ps2[:], lhsT=w3s[:, m, k * P:(k + 1) * P],
                                         rhs=xs[:, k, tok],
                                         start=(k == 0), stop=(k == DK - 1))
                    nc.vector.tensor_mul(out=ht[:, m], in0=sil[:], in1=ps2[:])
                    # Late staging, emitted right after its last blocker.
                    # Queue choice keeps each queue's head short: sem waits
                    # get bundled per queue by the scheduler, so a consumer
                    # can end up waiting on transfers emitted shortly after
                    # its own (keep those cheap or on other queues).
                    if blk == 0:
                        if m < MH - 1:
                            nc.sync.dma_start(out=w1s[:, m + 1], in_=w1v[m + 1])
                        if m < MH - 2:
                            nc.gpsimd.dma_start(out=w3s[:, m + 2], in_=w3v[m + 2])
                        if m >= 1:
                            nc.sync.dma_start(out=w2s[:, m - 1], in_=w2v[:, m - 1])
                        if m == MH - 1:
                            nc.sync.dma_start(out=w2s[:, 7], in_=w2v[:, 7])
                    if blk < NBLK - 1 and 1 <= m < 1 + DK:
                        stage_x(blk + 1, m - 1, nc.gpsimd if m % 2 else nc.sync)
                for j in range(DK):
                    js = slice(j * P, (j + 1) * P)
                    psy = psum.tile([P, BT], f32, tag="psy", bufs=2)
                    for m in range(MH):
                        nc.tensor.matmul(out=psy[:], lhsT=w2s[:, m, js],
                                         rhs=ht[:, m],
                                         start=(m == 0), stop=(m == MH - 1))
                    yt = act.tile([P, BT], bf16, tag="yt")
                    nc.vector.tensor_copy(out=yt[:], in_=psy[:])
                    # outputs: gpsimd is idle after the weight head, and
                    # using it keeps output completions out of the sem
                    # ranges x-staging consumers get bundled with. The last
                    # block's flush goes on the two fast HW queues.
                    if blk < NBLK - 1:
                        nc.gpsimd.dma_start(out=yT[js, tok], in_=yt[:])
                    elif j % 2 == 0:
                        nc.scalar.dma_start(out=yT[js, tok], in_=yt[:])
                    else:
                        nc.sync.dma_start(out=yT[js, tok], in_=yt[:])

    nc.compile()
    return nc


def _route(x2d, Wg, bg):
    """Replicate the reference router on host.

    Selection runs in float64 (agrees with the reference's fp32 jax
    selection whenever top-2/top-3 logit gaps exceed fp32 matmul noise,
    which holds with >10x margin on this distribution); the softmax over
    the two selected logits runs in fp32 like the reference.
    """
    logits64 = x2d.astype(np.float64) @ Wg.astype(np.float64) + bg.astype(np.float64)
    i1 = np.argmax(logits64, axis=1)
    r = np.arange(T)
    masked = logits64.copy()
    masked[r, i1] = -np.inf
    i2 = np.argmax(masked, axis=1)

    # fp32 logit values for the softmax (match reference arithmetic)
    logits32 = (x2d @ Wg + bg).astype(np.float32)
    v1 = logits32[r, i1]
    v2 = logits32[r, i2]
    # softmax over [v1, v2] with v1 >= v2 (fp32)
    e2 = np.exp((v2 - v1).astype(np.float32))
    p1 = (1.0 / (1.0 + e2)).astype(np.float32)
    p2 = (e2 / (1.0 + e2)).astype(np.float32)
    return i1, i2, p1, p2


def _ffn_host(x2d, idx, W1e, W3e, W2e):
    """Exact fp32 SwiGLU FFN for a small set of tokens (overflow path)."""
    z = x2d[idx] @ W1e
    h = (z / (1.0 + np.exp(-z))) * (x2d[idx] @ W3e)
    return h @ W2e


def kernel(x, Wg, bg, W1, W3, W2):
    global last_exec_time_ns
    _install_axon_trace_shim()
    _patch_upload_artifacts()
    from concourse.bass_utils import run_bass_kernel_spmd

    x = np.asarray(x, np.float32)
    Wg = np.asarray(Wg, np.float32)
    bg = np.asarray(bg, np.float32)
    W1 = np.asarray(W1, np.float32)
    W3 = np.asarray(W3, np.float32)
    W2 = np.asarray(W2, np.float32)

    B, S, _ = x.shape
    x2d = np.ascontiguousarray(x.reshape(T, D))

    i1, i2, p1, p2 = _route(x2d, Wg, bg)

    # Dispatch: build each expert's token list + gate weights. Tokens past
    # CAP (load-imbalance remainder) fall to the exact host path.
    idx_lists, gate_lists = [], []
    spill_lists = []
    for e in range(E):
        m1 = i1 == e
        m2 = i2 == e
        idx = np.concatenate([np.nonzero(m1)[0], np.nonzero(m2)[0]])
        g = np.concatenate([p1[m1], p2[m2]]).astype(np.float32)
        if len(idx) > CAP:
            # Spill the smallest-gate tokens: they matter least if anything
            # about the two paths' rounding ever differs.
            order = np.argsort(-g, kind="stable")
            idx, g = idx[order], g[order]
            spill_lists.append((idx[CAP:], g[CAP:]))
            idx, g = idx[:CAP], g[:CAP]
        else:
            spill_lists.append((idx[:0], g[:0]))
        idx_lists.append(idx)
        gate_lists.append(g)

    x2dT_bf = np.ascontiguousarray(x2d.T.astype(BF16))  # [D, T]
    in_maps = []
    for e in range(E):
        idx = idx_lists[e]
        xe = np.zeros((D, CAP), BF16)
        xe[:, : len(idx)] = x2dT_bf[:, idx]
        # m-major weight layout: [MH, P, DK*128] flattened to [MH*P, DK*128]
        w1m = np.ascontiguousarray(
            W1[e].astype(BF16).reshape(DK, P, MH, P).transpose(2, 1, 0, 3)
        ).reshape(MH * P, DK * P)
        w3m = np.ascontiguousarray(
            W3[e].astype(BF16).reshape(DK, P, MH, P).transpose(2, 1, 0, 3)
        ).reshape(MH * P, DK * P)
        in_maps.append({
            "xT": xe,
            "w1": w1m,
            "w3": w3m,
            "w2": np.ascontiguousarray(W2[e].astype(BF16)),
        })

    if "nc" not in _compiled:
        _compiled["nc"] = _build()
    nc = _compiled["nc"]

    trace = bool(os.environ.get("BASS_TRACE"))
    res = run_bass_kernel_spmd(nc, in_maps, list(range(N_CORES)), trace=trace)
    last_exec_time_ns = res.exec_time_ns
    globals()["last_results"] = res

    y = np.zeros((T, D), np.float32)
    for e in range(E):
        idx = idx_lists[e]
        n = len(idx)
        ye = np.asarray(res.results[e]["yT"])  # [D, CAP] bf16
        y[idx] += gate_lists[e][:, None] * ye[:, :n].T.astype(np.float32)
        sidx, sg = spill_lists[e]
        if len(sidx):
            y[sidx] += sg[:, None] * _ffn_host(x2d, sidx, W1[e], W3[e], W2[e])
    return y.reshape(B, S, D)
